# revision 1
# baseline (speedup 1.0000x reference)
"""LeViT-style attention block kernel for Trainium2 (8 NeuronCores, data-parallel over batch).

Reference computation (per batch b of 256, N=196 tokens, DIM=384):
  qkv = x @ qkv_w.T + qkv_b                      [196, 1152]
  q,k,v per head h (6): q,k [196,32], v [196,128]
  S = q @ k.T * 32^-0.5 + bias_h                 [196, 196]
  P = softmax(S, -1)
  O = P @ v  (concat heads -> [196, 768])
  A = hardswish(O)
  out = A @ proj_w.T + proj_b                    [196, 384]

Device mapping (per core: 32 batches = 6272 tokens):
  - host prepacks x.T (bf16), weight tiles, exp(bias) tables
  - qk computed transposed ([head-packed 128 rows, tokens]) on PE
  - v computed natural per batch ([tokens, 768])
  - S natural [n, m] -> exp on ACT -> *exp(bias) with fused row-sum on DVE
    -> P.T via PE matmul against diag(1/den) (transpose + normalize in one)
    -> O.T = v.T @ P.T on PE -> hardswish (ACT relu + fused min*mul on DVE)
    -> A.T accumulated into 128-token chunk tiles -> proj matmul -> out.
"""

import os
import sys

import numpy as np

sys.path.insert(0, "/opt/trn_rl_repo")

import ml_dtypes  # noqa: E402

import concourse.bass as bass  # noqa: E402
import concourse.tile as tile  # noqa: E402
from concourse import bacc, mybir  # noqa: E402
from concourse.bass_utils import run_bass_kernel_spmd  # noqa: E402

BF16 = mybir.dt.bfloat16
F32 = mybir.dt.float32
NPBF16 = ml_dtypes.bfloat16

N_CORES = 8
B, N, DIM = 256, 196, 384
KD, NH, D = 32, 6, 128  # key dim, heads, per-head v dim
DH = D * NH  # 768
RES = 14
SCALE = KD ** -0.5

AF = mybir.ActivationFunctionType
OP = mybir.AluOpType

# per-batch n/m tiling: token rows split 128 + 68
NT = [(0, 128), (128, 68)]

LAST_RESULT = {}  # test harness peeks at timing info here

# CoreSim rejects reads of never-written PSUM regions; the merged single-call
# exp / P.T-copy read (and discard) such garbage. EXACT_RANGES=True emits
# per-region calls instead — numerically identical — for simulator validation.
EXACT_RANGES = False
STAGES = 5  # debug bisect: 1=qkv only, 2=+S/exp/ttr, 3=+diag/PT, 4=+AV/hswish, 5=full


def _build_nc(bc):
    """Build the single-core program for bc batches (bc*196 tokens)."""
    T = bc * N
    assert T % 392 == 0
    nch = T // 392

    nc = bacc.Bacc("TRN2", target_bir_lowering=False, debug=False, num_devices=N_CORES)

    xt_d = nc.dram_tensor("xt", [DIM, T], BF16, kind="ExternalInput")
    wqk_d = nc.dram_tensor("wqk", [4, DIM, 128], BF16, kind="ExternalInput")
    bqk_d = nc.dram_tensor("bqk", [128, 4], F32, kind="ExternalInput")
    wv_d = nc.dram_tensor("wv", [DIM, DH], BF16, kind="ExternalInput")
    vb_d = nc.dram_tensor("vb", [128, DH], F32, kind="ExternalInput")
    wp_d = nc.dram_tensor("wp", [DH, DIM], BF16, kind="ExternalInput")
    pb_d = nc.dram_tensor("pb", [128, DIM], F32, kind="ExternalInput")
    pbr_d = nc.dram_tensor("pbr", [1, DIM], BF16, kind="ExternalInput")
    one_d = nc.dram_tensor("ones", [1, 128], BF16, kind="ExternalInput")
    eb_d = nc.dram_tensor("eb", [128, NH, 392], BF16, kind="ExternalInput")
    id_d = nc.dram_tensor("idm", [128, 128], BF16, kind="ExternalInput")
    hc_d = nc.dram_tensor("hc", [128, 1], F32, kind="ExternalInput")
    out_d = nc.dram_tensor("out", [T, DIM], F32, kind="ExternalOutput")

    with tile.TileContext(nc) as tc:
        with (
            tc.tile_pool(name="const", bufs=1) as cpool,
            tc.tile_pool(name="qkt", bufs=1) as qpool,
            tc.tile_pool(name="vtile", bufs=6) as vpool,
            tc.tile_pool(name="pu", bufs=3) as pupool,
            tc.tile_pool(name="pb2", bufs=8) as pbpool,
            tc.tile_pool(name="pts", bufs=3) as ptspool,
            tc.tile_pool(name="dg", bufs=3) as dgpool,
            tc.tile_pool(name="den", bufs=6) as denpool,
            tc.tile_pool(name="rt", bufs=3) as rpool,
            tc.tile_pool(name="atc", bufs=4) as atpool,
            tc.tile_pool(name="ob", bufs=3) as opool,
            tc.tile_pool(name="mmps", bufs=2, space="PSUM") as mmps,
            tc.tile_pool(name="sps", bufs=2, space="PSUM") as sps,
            tc.tile_pool(name="ptps", bufs=2, space="PSUM") as ptps,
            tc.tile_pool(name="otps", bufs=2, space="PSUM") as otps,
        ):
            # ---- constants into SBUF ----
            xt_t = cpool.tile([128, 3, T], BF16)
            for ct in range(3):
                nc.sync.dma_start(xt_t[:, ct, :], xt_d[128 * ct:128 * (ct + 1), :])
            wqk_t = cpool.tile([128, 4, 3, 128], BF16)
            for mt in range(4):
                for ct in range(3):
                    nc.sync.dma_start(wqk_t[:, mt, ct, :], wqk_d[mt, 128 * ct:128 * (ct + 1), :])
            wv_t = cpool.tile([128, 3, DH], BF16)
            for ct in range(3):
                nc.sync.dma_start(wv_t[:, ct, :], wv_d[128 * ct:128 * (ct + 1), :])
            wp_t = cpool.tile([128, NH, DIM], BF16)
            for kt in range(NH):
                nc.sync.dma_start(wp_t[:, kt, :], wp_d[128 * kt:128 * (kt + 1), :])
            eb_t = cpool.tile([128, NH, 392], BF16)
            nc.sync.dma_start(eb_t[:], eb_d[:])
            bqk_t = cpool.tile([128, 4], F32)
            nc.sync.dma_start(bqk_t[:], bqk_d[:])
            vb_t = cpool.tile([128, DH], F32)
            nc.sync.dma_start(vb_t[:], vb_d[:])
            pb_t = cpool.tile([128, DIM], F32)
            nc.sync.dma_start(pb_t[:], pb_d[:])
            pbr_t = cpool.tile([1, DIM], BF16)
            nc.sync.dma_start(pbr_t[:], pbr_d[:])
            one_t = cpool.tile([1, 128], BF16)
            nc.sync.dma_start(one_t[:], one_d[:])
            id_t = cpool.tile([128, 128], BF16)
            nc.sync.dma_start(id_t[:], id_d[:])
            hc_t = cpool.tile([128, 1], F32)
            nc.sync.dma_start(hc_t[:], hc_d[:])

            # ---- qk^T phase: tQ/tK hold q/k of heads 0-3 at partition 32h;
            #      tQ2/tK2 hold heads 4-5 at partition 32(h-4) (rows 0:64) ----
            tQ = qpool.tile([128, T], BF16, tag="tq")
            tK = qpool.tile([128, T], BF16, tag="tk")
            tQ2 = qpool.tile([128, T], BF16, tag="tq2")
            tK2 = qpool.tile([128, T], BF16, tag="tk2")
            qk_dests = [(0, tQ, 128), (1, tK, 128), (2, tQ2, 64), (3, tK2, 64)]
            for ch in range(nch):
                c0 = 392 * ch
                for mt, dest, msz in qk_dests:
                    ps = mmps.tile([128, 392], F32, tag="mm")
                    for ct in range(3):
                        nc.tensor.matmul(
                            ps[0:msz, :],
                            wqk_t[:, mt, ct, 0:msz],
                            xt_t[:, ct, c0:c0 + 392],
                            start=(ct == 0), stop=(ct == 2),
                        )
                    nc.scalar.activation(
                        dest[0:msz, c0:c0 + 392], ps[0:msz, :], AF.Identity,
                        bias=bqk_t[0:msz, mt:mt + 1], scale=1.0,
                    )

            for b in range(bc):
                b0 = b * N
                # ---- v natural [tokens, 768] for this batch ----
                v_t = vpool.tile([128, 2, DH], BF16, tag="v")
                for nt, (r0, nsz) in enumerate(NT):
                    for half in range(2):
                        h0 = 384 * half
                        ps = mmps.tile([128, 392], F32, tag="mm")
                        for ct in range(3):
                            nc.tensor.matmul(
                                ps[0:nsz, 0:384],
                                xt_t[:, ct, b0 + r0:b0 + r0 + nsz],
                                wv_t[:, ct, h0:h0 + 384],
                                start=(ct == 0), stop=(ct == 2),
                            )
                        nc.vector.tensor_add(
                            v_t[0:nsz, nt, h0:h0 + 384], ps[0:nsz, 0:384],
                            vb_t[0:nsz, h0:h0 + 384],
                        )

                if STAGES < 2:
                    continue
                # ---- stage 1 per head: S, exp, *exp(bias) with fused
                #      row-sum -> den ----
                den = denpool.tile([128, 2 * NH], F32, tag="den")
                nc.gpsimd.memset(den[:], 1.0)
                p_bs = []
                for h in range(NH):
                    if h < 4:
                        qsrc, ksrc, base = tQ, tK, 32 * h
                    else:
                        qsrc, ksrc, base = tQ2, tK2, 32 * (h - 4)
                    s_ps = sps.tile([128, 392], F32, tag="s")
                    for nt, (r0, nsz) in enumerate(NT):
                        nc.tensor.matmul(
                            s_ps[0:nsz, 196 * nt:196 * nt + 196],
                            qsrc[base:base + 32, b0 + r0:b0 + r0 + nsz],
                            ksrc[base:base + 32, b0:b0 + 196],
                            start=True, stop=True,
                            tile_position=(base, 0),
                        )
                    p_u = pupool.tile([128, 392], BF16, tag="pu", name=f"pu{h}")
                    if EXACT_RANGES:
                        for nt, (r0, nsz) in enumerate(NT):
                            reg = slice(196 * nt, 196 * nt + 196)
                            nc.scalar.activation(p_u[0:nsz, reg], s_ps[0:nsz, reg], AF.Exp)
                    else:
                        nc.scalar.activation(p_u[:, :], s_ps[:, :], AF.Exp)
                    p_b = pbpool.tile([128, 392], BF16, tag="pb", name=f"pb{h}")
                    for nt, (r0, nsz) in enumerate(NT):
                        reg = slice(196 * nt, 196 * nt + 196)
                        nc.vector.scalar_tensor_tensor(
                            out=p_b[0:nsz, reg], in0=p_u[0:nsz, reg], scalar=0.0,
                            in1=eb_t[0:nsz, h, reg], op0=OP.bypass, op1=OP.mult,
                            accum_out=den[0:nsz, 2 * h + nt:2 * h + nt + 1],
                        )
                    p_bs.append(p_b)
                rho = denpool.tile([128, 2 * NH], F32, tag="rho")
                nc.vector.reciprocal(rho[:, :], den[:, :])

                # ---- stage 2: diag + P.T per head; O.T packed two heads
                #      per PSUM bank so hardswish runs once per head-pair ----
                if STAGES < 3:
                    continue
                at_b = atpool.tile([128, NH * 196], BF16, tag="at")
                for hp in range(NH // 2):
                    ot_ps = otps.tile([128, 392], F32, tag="ot")
                    for hh in range(2):
                        h = 2 * hp + hh
                        p_b = p_bs[h]
                        dg = dgpool.tile([128, 196], BF16, tag="dg")
                        nc.gpsimd.tensor_scalar_mul(
                            dg[0:128, 0:128], id_t[:, :], rho[0:128, 2 * h:2 * h + 1])
                        nc.gpsimd.tensor_scalar_mul(
                            dg[0:68, 128:196], id_t[0:68, 0:68], rho[0:68, 2 * h + 1:2 * h + 2])
                        pt_ps = ptps.tile([128, 392], F32, tag="pt")
                        for mt, (m0, msz) in enumerate(NT):
                            for nt, (n0, nsz) in enumerate(NT):
                                nc.tensor.matmul(
                                    pt_ps[0:msz, 196 * mt + n0:196 * mt + n0 + nsz],
                                    p_b[0:nsz, 196 * nt + m0:196 * nt + m0 + msz],
                                    dg[0:nsz, 128 * nt:128 * nt + nsz],
                                    start=True, stop=True,
                                )
                        if STAGES < 4:
                            continue
                        pt_sb = ptspool.tile([128, 392], BF16, tag="pts")
                        if EXACT_RANGES:
                            nc.scalar.activation(pt_sb[0:128, 0:196], pt_ps[0:128, 0:196], AF.Copy)
                            nc.scalar.activation(pt_sb[0:68, 196:392], pt_ps[0:68, 196:392], AF.Copy)
                        else:
                            nc.scalar.activation(pt_sb[:, :], pt_ps[:, :], AF.Copy)
                        # ---- O.T [128, 196] = v.T @ P.T ----
                        for kt, (k0, ksz) in enumerate(NT):
                            nc.tensor.matmul(
                                ot_ps[:, 196 * hh:196 * hh + 196],
                                v_t[0:ksz, kt, 128 * h:128 * h + 128],
                                pt_sb[0:ksz, 196 * kt:196 * kt + 196],
                                start=(kt == 0), stop=(kt == 1),
                            )
                    if STAGES < 4:
                        continue
                    # ---- 6*hardswish(O) = O*(clamp(O,-3,3)+3) per pair;
                    #      the /6 is folded into the projection weights ----
                    m_t = rpool.tile([128, 392], BF16, tag="m")
                    nc.vector.tensor_scalar(
                        out=m_t[:, :], in0=ot_ps[:, :],
                        scalar1=3.0, scalar2=-3.0, op0=OP.min, op1=OP.max,
                    )
                    nc.vector.scalar_tensor_tensor(
                        out=at_b[:, 392 * hp:392 * hp + 392], in0=m_t[:, :], scalar=3.0,
                        in1=ot_ps[:, :], op0=OP.add, op1=OP.mult,
                    )

                # ---- proj for this batch ----
                if STAGES < 5:
                    continue
                for nt, (r0, nsz) in enumerate(NT):
                    ps = mmps.tile([128, 392], F32, tag="mm")
                    nc.tensor.matmul(
                        ps[0:nsz, 0:384], one_t[0:1, 0:nsz], pbr_t[0:1, :],
                        start=True, stop=False,
                    )
                    for kt in range(NH):
                        nc.tensor.matmul(
                            ps[0:nsz, 0:384],
                            at_b[:, 196 * kt + r0:196 * kt + r0 + nsz], wp_t[:, kt, :],
                            start=False, stop=(kt == NH - 1),
                        )
                    ob = opool.tile([128, DIM], F32, tag="ob")
                    nc.scalar.activation(ob[0:nsz, :], ps[0:nsz, 0:384], AF.Copy)
                    nc.sync.dma_start(out_d[b0 + r0:b0 + r0 + nsz, :], ob[0:nsz, :])

    nc.finalize()  # run Bacc passes (reg alloc, wait splitting) before walrus
    return nc


def _host_pack(x, qkv_w, qkv_b, proj_w, proj_b, attn_biases, bias_idxs, bc):
    """Build the common (replicated) input map and per-core xt slices."""
    w = np.asarray(qkv_w, np.float32).reshape(NH, 192, DIM)
    bia = np.asarray(qkv_b, np.float32).reshape(NH, 192)
    qw = w[:, 0:KD, :] * SCALE          # [6, 32, 384]
    kw = w[:, KD:2 * KD, :]
    vw = w[:, 2 * KD:, :]               # [6, 128, 384]
    qb = bia[:, 0:KD] * SCALE
    kb = bia[:, KD:2 * KD]
    vb = bia[:, 2 * KD:]

    wqk = np.zeros((4, DIM, 128), np.float32)
    wqk[0, :, :] = qw[0:4].reshape(128, DIM).T
    wqk[1, :, :] = kw[0:4].reshape(128, DIM).T
    wqk[2, :, 0:64] = qw[4:6].reshape(64, DIM).T
    wqk[3, :, 0:64] = kw[4:6].reshape(64, DIM).T
    bqk = np.zeros((128, 4), np.float32)
    bqk[:, 0] = qb[0:4].reshape(128)
    bqk[:, 1] = kb[0:4].reshape(128)
    bqk[0:64, 2] = qb[4:6].reshape(64)
    bqk[0:64, 3] = kb[4:6].reshape(64)

    wv = vw.reshape(DH, DIM).T.copy()          # [384, 768], head h at cols 128h
    vbt = np.tile(vb.reshape(1, DH), (128, 1)).astype(np.float32)
    # device computes 6*hardswish; absorb the 1/6 into the projection weights
    wp = (np.asarray(proj_w, np.float32).T / 6.0).copy()  # [768, 384]
    pbt = np.tile(np.asarray(proj_b, np.float32).reshape(1, DIM), (128, 1))

    bmat = np.asarray(attn_biases, np.float32)[:, np.asarray(bias_idxs)]  # [6,196,196]
    ebp = np.zeros((128, NH, 392), np.float32)
    eb = np.exp(bmat)
    for h in range(NH):
        ebp[0:128, h, 0:196] = eb[h, 0:128, :]
        ebp[0:68, h, 196:392] = eb[h, 128:196, :]

    common = {
        "pbr": np.asarray(proj_b, np.float32).reshape(1, DIM).astype(NPBF16),
        "ones": np.ones((1, 128), NPBF16),
        "wqk": wqk.astype(NPBF16),
        "bqk": bqk,
        "wv": wv.astype(NPBF16),
        "vb": vbt,
        "wp": wp.astype(NPBF16),
        "pb": pbt,
        "eb": ebp.astype(NPBF16),
        "idm": np.eye(128, dtype=NPBF16),
        "hc": np.full((128, 1), 0.5, np.float32),
    }

    x = np.asarray(x, np.float32)
    n_cores = x.shape[0] // bc
    xts = []
    for c in range(n_cores):
        xc = x[bc * c:bc * (c + 1)].reshape(bc * N, DIM)
        xts.append(np.ascontiguousarray(xc.T).astype(NPBF16))
    return common, xts


_NC_CACHE = {}


def kernel(x, qkv_w, qkv_b, proj_w, proj_b, attn_biases, bias_idxs):
    bc = B // N_CORES
    if bc not in _NC_CACHE:
        _NC_CACHE[bc] = _build_nc(bc)
    nc = _NC_CACHE[bc]
    common, xts = _host_pack(x, qkv_w, qkv_b, proj_w, proj_b, attn_biases, bias_idxs, bc)
    in_maps = [dict(common, xt=xts[c]) for c in range(N_CORES)]
    trace = bool(int(os.environ.get("KT_TRACE", "0")))
    res = run_bass_kernel_spmd(nc, in_maps, list(range(N_CORES)), trace=trace)
    LAST_RESULT["exec_time_ns"] = res.exec_time_ns
    LAST_RESULT["mean_exec_time_ns"] = res.mean_exec_time_ns
    outs = [res.results[c]["out"].reshape(bc, N, DIM) for c in range(N_CORES)]
    return np.concatenate(outs, axis=0).astype(np.float32)



# revision 35
# speedup vs baseline: 207.1235x; 207.1235x over previous
"""LeViT-style attention block kernel for Trainium2 (8 NeuronCores, data-parallel over batch).

Reference computation (per batch b of 256, N=196 tokens, DIM=384):
  qkv = x @ qkv_w.T + qkv_b                      [196, 1152]
  q,k,v per head h (6): q,k [196,32], v [196,128]
  S = q @ k.T * 32^-0.5 + bias_h                 [196, 196]
  P = softmax(S, -1)
  O = P @ v  (concat heads -> [196, 768])
  A = hardswish(O)
  out = A @ proj_w.T + proj_b                    [196, 384]

Device mapping (per core: 32 batches = 6272 tokens):
  - host prepacks x.T (bf16), weight tiles, exp(bias)^T tables
  - q/k computed transposed ([head-packed 128 rows, tokens]) on PE
  - v computed natural per batch ([tokens, 768])
  - S^T[m,n] = k^T q directly on PE (keys m on partitions, queries n free)
    -> exp on ACT -> *exp(bias)^T on DVE = P-hat^T
    -> den[n] = ones-column matmul over P-hat^T rows (PE), rho = 1/den (DVE)
    -> O^T = v^T @ P-hat^T on PE (v natural is already the right stationary)
    -> normalize O^T by rho via partition-broadcast AP + hardswish on DVE
    -> A^T per head -> proj matmul -> out.
  No transpose matmuls and no GpSimd work (the v1 kernel spent 70% of the
  span in GpSimd diag builds for a P^T-via-matmul transpose).
"""

import os
import sys

import numpy as np

sys.path.insert(0, "/opt/trn_rl_repo")

import ml_dtypes  # noqa: E402

import concourse.bass as bass  # noqa: E402
import concourse.tile as tile  # noqa: E402
from concourse import bacc, mybir  # noqa: E402
from concourse.bass_utils import run_bass_kernel_spmd  # noqa: E402

BF16 = mybir.dt.bfloat16
F32 = mybir.dt.float32
F8 = mybir.dt.float8e4
NPBF16 = ml_dtypes.bfloat16
NPF8 = mybir.dt.np(F8)
DR = mybir.MatmulPerfMode.DoubleRow

N_CORES = 8
B, N, DIM = 256, 196, 384
KD, NH, D = 32, 6, 128  # key dim, heads, per-head v dim
DH = D * NH  # 768
RES = 14
SCALE = KD ** -0.5

AF = mybir.ActivationFunctionType
OP = mybir.AluOpType

# per-batch key/query token tiling: 196 rows split 128 + 68
NT = [(0, 128), (128, 68)]

LAST_RESULT = {}  # test harness peeks at timing info here


def _build_nc(bc):
    """Build the single-core program for bc batches (bc*196 tokens)."""
    T = bc * N
    assert T % 392 == 0
    nch = T // 392

    nc = bacc.Bacc("TRN2", target_bir_lowering=False, debug=False, num_devices=N_CORES)

    xt_d = nc.dram_tensor("xt", [DIM, T], BF16, kind="ExternalInput")
    wqk_d = nc.dram_tensor("wqk", [4, DIM, 128], BF16, kind="ExternalInput")
    bqk_d = nc.dram_tensor("bqk", [128, 4], F32, kind="ExternalInput")
    wv_d = nc.dram_tensor("wv", [DIM, DH], BF16, kind="ExternalInput")
    vb_d = nc.dram_tensor("vb", [128, DH], F32, kind="ExternalInput")
    vbc_d = nc.dram_tensor("vbc", [128, NH], F32, kind="ExternalInput")
    wp_d = nc.dram_tensor("wp", [DH, DIM], BF16, kind="ExternalInput")
    pbr_d = nc.dram_tensor("pbr", [1, DIM], BF16, kind="ExternalInput")
    one_d = nc.dram_tensor("ones", [1, 128], BF16, kind="ExternalInput")
    onem_d = nc.dram_tensor("onem", [128, 128], BF16, kind="ExternalInput")
    ebt_d = nc.dram_tensor("ebt", [128, NH // 2, 1024], BF16, kind="ExternalInput")
    out_d = nc.dram_tensor("out", [T, DIM], F32, kind="ExternalOutput")

    with tile.TileContext(nc) as tc:
        with (
            tc.tile_pool(name="const", bufs=1) as cpool,
            tc.tile_pool(name="qkt", bufs=1) as qpool,
            tc.tile_pool(name="vtile", bufs=4) as vpool,
            tc.tile_pool(name="pu", bufs=3) as pupool,
            tc.tile_pool(name="pb2", bufs=4) as pbpool,
            tc.tile_pool(name="rho", bufs=4) as rpool,
            tc.tile_pool(name="onm", bufs=3) as onpool,
            tc.tile_pool(name="atc", bufs=3) as atpool,
            tc.tile_pool(name="ob", bufs=3) as opool,
            tc.tile_pool(name="mmps", bufs=2, space="PSUM") as mmps,
            tc.tile_pool(name="sps", bufs=2, space="PSUM") as sps,
            tc.tile_pool(name="otps", bufs=2, space="PSUM") as otps,
        ):
            # ---- constants into SBUF; small tensors first so the first
            #      qkv chunk isn't queued behind megabyte-scale DMAs, and
            #      xt split in halves so chunk 0 lands early ----
            bqk_t = cpool.tile([128, 4], F32)
            nc.sync.dma_start(bqk_t[:], bqk_d[:])
            vb_t = cpool.tile([128, DH], F32)
            nc.sync.dma_start(vb_t[:], vb_d[:])
            vbc_t = cpool.tile([128, NH], F32)
            nc.sync.dma_start(vbc_t[:], vbc_d[:])
            pbr_t = cpool.tile([1, DIM], BF16)
            nc.sync.dma_start(pbr_t[:], pbr_d[:])
            one_t = cpool.tile([1, 128], BF16)
            nc.sync.dma_start(one_t[:], one_d[:])
            onem_t = cpool.tile([128, 128], BF16)
            nc.sync.dma_start(onem_t[:], onem_d[:])
            wqk_t = cpool.tile([128, 4, 3, 128], BF16)
            for mt in range(4):
                for ct in range(3):
                    nc.sync.dma_start(wqk_t[:, mt, ct, :], wqk_d[mt, 128 * ct:128 * (ct + 1), :])
            xt_t = cpool.tile([128, 3, T], BF16)
            for hf, (f0, f1) in enumerate([(0, T // 4), (T // 4, T)]):
                for ct in range(3):
                    nc.sync.dma_start(
                        xt_t[:, ct, f0:f1], xt_d[128 * ct:128 * (ct + 1), f0:f1])
            wv_t = cpool.tile([128, 3, DH], BF16)
            for ct in range(3):
                nc.sync.dma_start(wv_t[:, ct, :], wv_d[128 * ct:128 * (ct + 1), :])
            ebt_t = cpool.tile([128, NH // 2, 1024], BF16)
            nc.sync.dma_start(ebt_t[:], ebt_d[:])
            wp_t = cpool.tile([128, NH, DIM], BF16)
            for kt in range(NH):
                nc.sync.dma_start(wp_t[:, kt, :], wp_d[128 * kt:128 * (kt + 1), :])

            # ---- q/k^T phase: tQ/tK hold q/k of heads 0-3 at partition 32h;
            #      tQ2/tK2 hold heads 4-5 at partition 32(h-4) (rows 0:64) ----
            tQ = qpool.tile([128, T], BF16, tag="tq")
            tK = qpool.tile([128, T], BF16, tag="tk")
            tQ2 = qpool.tile([128, T], BF16, tag="tq2")
            tK2 = qpool.tile([128, T], BF16, tag="tk2")
            qk_dests = [(0, tQ, 128), (1, tK, 128), (2, tQ2, 64), (3, tK2, 64)]
            for ch in range(nch):
                c0 = 392 * ch
                for mt, dest, msz in qk_dests:
                    ps = mmps.tile([128, 392], F32, tag="mm")
                    for ct in range(3):
                        nc.tensor.matmul(
                            ps[0:msz, :],
                            wqk_t[:, mt, ct, 0:msz],
                            xt_t[:, ct, c0:c0 + 392],
                            start=(ct == 0), stop=(ct == 2),
                        )
                    nc.scalar.activation(
                        dest[0:msz, c0:c0 + 392], ps[0:msz, :], AF.Identity,
                        bias=bqk_t[0:msz, mt:mt + 1], scale=1.0,
                    )

            def emit_s(hp, b0):
                """S^T matmuls for one head pair; head hh at column 512*hh of a
                bank-padded [128,1024] PSUM tile (m-tile mt at 512*hh+196*mt).
                Pairs 0 and 1 are emitted back-to-back so their 4 distinct
                tile_position row groups can run concurrently in the PE."""
                s2 = sps.tile([128, 1024], F32, tag="s")
                for hh in range(2):
                    h = 2 * hp + hh
                    if h < 4:
                        qsrc, ksrc, base = tQ, tK, 32 * h
                    else:
                        qsrc, ksrc, base = tQ2, tK2, 32 * (h - 4)
                    for mt, (m0, msz) in enumerate(NT):
                        nc.tensor.matmul(
                            s2[0:msz, 512 * hh + 196 * mt:512 * hh + 196 * mt + 196],
                            ksrc[base:base + 32, b0 + m0:b0 + m0 + msz],
                            qsrc[base:base + 32, b0:b0 + 196],
                            start=True, stop=True,
                            tile_position=(base, 0),
                        )
                return s2

            def process_pair(hp, s2, v_t, at_b):
                # strided [128, 2, 392] views skip the 392:512 pad columns
                # (pads are never read downstream — den/OT use exact regions)
                s2v = s2[:].rearrange("p (h c) -> p h c", h=2)[:, :, 0:392]
                p_u = pupool.tile([128, 1024], BF16, tag="pu")
                p_uv = p_u[:].rearrange("p (h c) -> p h c", h=2)[:, :, 0:392]
                nc.scalar.activation(p_uv, s2v, AF.Exp)
                # P-hat^T = exp(S^T) * exp(bias)^T
                pb2 = pbpool.tile([128, 1024], BF16, tag="pb")
                ebv = ebt_t[:, hp, :].rearrange("p (h c) -> p h c", h=2)[:, :, 0:392]
                nc.vector.tensor_mul(
                    pb2[:].rearrange("p (h c) -> p h c", h=2)[:, :, 0:392],
                    p_uv, ebv)
                # den[n] = sum_m P-hat^T[m, n] via all-ones matmul; the
                # [msz,128] ones stationary broadcasts den to all 128
                # partitions so the normalize multiply needs no
                # partition-broadcast AP (DVE requires nonzero step).
                # One matmul per m-tile covers BOTH heads via a strided
                # rhs AP [msz, 2, 196] (head stride 512).
                # den lands in ot_ps first; the O^T matmuls overwrite it
                # after the reciprocal is taken (WAR order via Tile).
                ot_ps = otps.tile([128, 392], F32, tag="ot")
                pb2v = pb2[:].rearrange("p (h c) -> p h c", h=2)
                for mt, (m0, msz) in enumerate(NT):
                    nc.tensor.matmul(
                        ot_ps[:, :],
                        onem_t[0:msz, :],
                        pb2v[0:msz, :, 196 * mt:196 * mt + 196],
                        start=(mt == 0), stop=(mt == 1),
                    )
                rho = rpool.tile([128, 392], F32, tag="rho")
                nc.vector.reciprocal_approx_fast(rho[:, :], ot_ps[:, :])
                # ---- O^T [128, 196] per head = v^T @ P-hat^T ----
                for hh in range(2):
                    h = 2 * hp + hh
                    for mt, (m0, msz) in enumerate(NT):
                        nc.tensor.matmul(
                            ot_ps[:, 196 * hh:196 * hh + 196],
                            v_t[0:msz, mt, 128 * h:128 * h + 128],
                            pb2[0:msz, 512 * hh + 196 * mt:512 * hh + 196 * mt + 196],
                            start=(mt == 0), stop=(mt == 1),
                        )
                # ---- normalize by rho; heads 3-5 add the v-bias here
                #      (per-partition in O^T layout; softmax rows sum to 1
                #      so O = P(xWv)/den + bv) ----
                o_n = onpool.tile([128, 392], BF16, tag="on")
                nc.vector.tensor_mul(o_n[:, :], ot_ps[:, :], rho[:, :])
                for hh in range(2):
                    h = 2 * hp + hh
                    if h >= 3:
                        reg = slice(196 * hh, 196 * hh + 196)
                        nc.vector.tensor_scalar_add(
                            o_n[:, reg], o_n[:, reg], vbc_t[:, h:h + 1])
                # ---- 6*hardswish(O) = O*(clamp(O,-3,3)+3); the /6 is
                #      folded into the projection weights ----
                m_t = onpool.tile([128, 392], BF16, tag="mt")
                nc.vector.tensor_scalar(
                    out=m_t[:, :], in0=o_n[:, :],
                    scalar1=3.0, scalar2=-3.0, op0=OP.min, op1=OP.max,
                )
                nc.vector.tensor_scalar_add(m_t[:, :], m_t[:, :], 3.0)
                nc.vector.tensor_mul(
                    at_b[:, 392 * hp:392 * hp + 392], m_t[:, :], o_n[:, :])

            def emit_proj(b, at_b):
                b0 = b * N
                for nt, (r0, nsz) in enumerate(NT):
                    ps = mmps.tile([128, 392], F32, tag="mm")
                    nc.tensor.matmul(
                        ps[0:nsz, 0:384], one_t[0:1, 0:nsz], pbr_t[0:1, :],
                        start=True, stop=False,
                    )
                    for kt in range(NH):
                        nc.tensor.matmul(
                            ps[0:nsz, 0:384],
                            at_b[:, 196 * kt + r0:196 * kt + r0 + nsz], wp_t[:, kt, :],
                            start=False, stop=(kt == NH - 1),
                        )
                    ob = opool.tile([128, DIM], F32, tag="ob")
                    nc.scalar.activation(ob[0:nsz, :], ps[0:nsz, 0:384], AF.Copy)
                    nc.sync.dma_start(out_d[b0 + r0:b0 + r0 + nsz, :], ob[0:nsz, :])

            for b in range(bc):
                b0 = b * N
                # ---- v natural [tokens, 768] for this batch; low half
                #      evacuated on DVE (with bias add), high half on ACT
                #      (bias for heads 3-5 folded into o_n above) ----
                v_t = vpool.tile([128, 2, DH], BF16, tag="v")
                for nt, (r0, nsz) in enumerate(NT):
                    for half in range(2):
                        h0 = 384 * half
                        ps = mmps.tile([128, 392], F32, tag="mm")
                        for ct in range(3):
                            nc.tensor.matmul(
                                ps[0:nsz, 0:384],
                                xt_t[:, ct, b0 + r0:b0 + r0 + nsz],
                                wv_t[:, ct, h0:h0 + 384],
                                start=(ct == 0), stop=(ct == 2),
                            )
                        if half == 0:
                            nc.vector.tensor_add(
                                v_t[0:nsz, nt, 0:384], ps[0:nsz, 0:384],
                                vb_t[0:nsz, 0:384],
                            )
                        else:
                            nc.scalar.activation(
                                v_t[0:nsz, nt, 384:768], ps[0:nsz, 0:384], AF.Copy)

                at_b = atpool.tile([128, NH * 196], BF16, tag="at")
                s2a = emit_s(0, b0)
                s2b = emit_s(1, b0)
                # proj for the PREVIOUS batch goes here: its 14 N=384 matmuls
                # fill the PE bubble while this batch's exp/bias-mul run
                if b > 0:
                    emit_proj(b - 1, prev_at)
                process_pair(0, s2a, v_t, at_b)
                process_pair(1, s2b, v_t, at_b)
                s2c = emit_s(2, b0)
                process_pair(2, s2c, v_t, at_b)
                prev_at = at_b
            emit_proj(bc - 1, prev_at)

    nc.finalize()  # run Bacc passes (reg alloc, wait splitting) before walrus
    return nc


def _host_pack(x, qkv_w, qkv_b, proj_w, proj_b, attn_biases, bias_idxs, bc):
    """Build the common (replicated) input map and per-core xt slices."""
    w = np.asarray(qkv_w, np.float32).reshape(NH, 192, DIM)
    bia = np.asarray(qkv_b, np.float32).reshape(NH, 192)
    qw = w[:, 0:KD, :] * SCALE          # [6, 32, 384]
    kw = w[:, KD:2 * KD, :]
    vw = w[:, 2 * KD:, :]               # [6, 128, 384]
    qb = bia[:, 0:KD] * SCALE
    kb = bia[:, KD:2 * KD]
    vb = bia[:, 2 * KD:]

    wqk = np.zeros((4, DIM, 128), np.float32)
    wqk[0, :, :] = qw[0:4].reshape(128, DIM).T
    wqk[1, :, :] = kw[0:4].reshape(128, DIM).T
    wqk[2, :, 0:64] = qw[4:6].reshape(64, DIM).T
    wqk[3, :, 0:64] = kw[4:6].reshape(64, DIM).T
    bqk = np.zeros((128, 4), np.float32)
    bqk[:, 0] = qb[0:4].reshape(128)
    bqk[:, 1] = kb[0:4].reshape(128)
    bqk[0:64, 2] = qb[4:6].reshape(64)
    bqk[0:64, 3] = kb[4:6].reshape(64)

    wv = vw.reshape(DH, DIM).T.copy()          # [384, 768], head h at cols 128h
    vbt = np.tile(vb.reshape(1, DH), (128, 1)).astype(np.float32)
    vbc = np.ascontiguousarray(vb.T)           # [128, 6], col h = head h's bias
    # device computes 6*hardswish; absorb the 1/6 into the projection weights
    wp = (np.asarray(proj_w, np.float32).T / 6.0).copy()  # [768, 384]

    bmat = np.asarray(attn_biases, np.float32)[:, np.asarray(bias_idxs)]  # [6,196,196]
    ebp = np.zeros((128, NH // 2, 1024), np.float32)
    ebT = np.exp(np.transpose(bmat, (0, 2, 1)))  # [6, m, n]
    for h in range(NH):
        hp, hh = divmod(h, 2)
        ebp[0:128, hp, 512 * hh + 0:512 * hh + 196] = ebT[h, 0:128, :]
        ebp[0:68, hp, 512 * hh + 196:512 * hh + 392] = ebT[h, 128:196, :]

    common = {
        "pbr": np.asarray(proj_b, np.float32).reshape(1, DIM).astype(NPBF16),
        "ones": np.ones((1, 128), NPBF16),
        "onem": np.ones((128, 128), NPBF16),
        "wqk": wqk.astype(NPBF16),
        "bqk": bqk,
        "wv": wv.astype(NPBF16),
        "vb": vbt,
        "vbc": vbc.astype(np.float32),
        "wp": wp.astype(NPBF16),
        "ebt": ebp.astype(NPBF16),
    }

    x = np.asarray(x, np.float32)
    n_cores = x.shape[0] // bc
    xts = []
    for c in range(n_cores):
        xc = x[bc * c:bc * (c + 1)].reshape(bc * N, DIM)
        xts.append(np.ascontiguousarray(xc.T).astype(NPBF16))
    return common, xts


_NC_CACHE = {}


def kernel(x, qkv_w, qkv_b, proj_w, proj_b, attn_biases, bias_idxs):
    bc = B // N_CORES
    if bc not in _NC_CACHE:
        _NC_CACHE[bc] = _build_nc(bc)
    nc = _NC_CACHE[bc]
    common, xts = _host_pack(x, qkv_w, qkv_b, proj_w, proj_b, attn_biases, bias_idxs, bc)
    in_maps = [dict(common, xt=xts[c]) for c in range(N_CORES)]
    trace = bool(int(os.environ.get("KT_TRACE", "0")))
    res = run_bass_kernel_spmd(nc, in_maps, list(range(N_CORES)), trace=trace)
    LAST_RESULT["exec_time_ns"] = res.exec_time_ns
    LAST_RESULT["mean_exec_time_ns"] = res.mean_exec_time_ns
    outs = [res.results[c]["out"].reshape(bc, N, DIM) for c in range(N_CORES)]
    return np.concatenate(outs, axis=0).astype(np.float32)


# revision 44
# speedup vs baseline: 211.5041x; 1.0211x over previous
"""LeViT-style attention block kernel for Trainium2 (8 NeuronCores, data-parallel over batch).

Reference computation (per batch b of 256, N=196 tokens, DIM=384):
  qkv = x @ qkv_w.T + qkv_b                      [196, 1152]
  q,k,v per head h (6): q,k [196,32], v [196,128]
  S = q @ k.T * 32^-0.5 + bias_h                 [196, 196]
  P = softmax(S, -1)
  O = P @ v  (concat heads -> [196, 768])
  A = hardswish(O)
  out = A @ proj_w.T + proj_b                    [196, 384]

Device mapping (per core: 32 batches = 6272 tokens):
  - host prepacks x.T (bf16), weight tiles, exp(bias)^T tables
  - q/k computed transposed ([head-packed 128 rows, tokens]) on PE,
    interleaved into the batch loop (2 chunks of runway) as PE filler
  - v computed natural per batch ([tokens, 768])
  - S^T[m,n] = k^T q directly on PE (keys m on partitions, queries n free)
    -> exp on ACT -> *exp(bias)^T on DVE = P-hat^T
    -> den[n] = all-ones matmul over P-hat^T rows (PE; [msz,128] ones
       stationary broadcasts den to all partitions), rho = 1/den via
       reciprocal_approx_fast on DVE
    -> O^T = v^T @ P-hat^T on PE (v natural is already the right stationary)
    -> normalize O^T by rho + hardswish on DVE; v-bias of heads 3-5 folded
       here per-partition (softmax rows sum to 1, so O = P(xWv)/den + bv)
    -> A^T per head -> proj matmul (lagged one batch as PE filler) -> out.
  No transpose matmuls and no GpSimd work (the v1 kernel spent 70% of the
  span in GpSimd diag builds for a P^T-via-matmul transpose: 885us; this
  version measures ~345-358us NTFF HW exec across runs).
"""

import os
import sys

import numpy as np

sys.path.insert(0, "/opt/trn_rl_repo")

import ml_dtypes  # noqa: E402

import concourse.bass as bass  # noqa: E402
import concourse.tile as tile  # noqa: E402
from concourse import bacc, mybir  # noqa: E402
from concourse.bass_utils import run_bass_kernel_spmd  # noqa: E402

BF16 = mybir.dt.bfloat16
F32 = mybir.dt.float32
F8 = mybir.dt.float8e4
NPBF16 = ml_dtypes.bfloat16
NPF8 = mybir.dt.np(F8)
DR = mybir.MatmulPerfMode.DoubleRow

N_CORES = 8
B, N, DIM = 256, 196, 384
KD, NH, D = 32, 6, 128  # key dim, heads, per-head v dim
DH = D * NH  # 768
RES = 14
SCALE = KD ** -0.5

AF = mybir.ActivationFunctionType
OP = mybir.AluOpType

# per-batch key/query token tiling: 196 rows split 128 + 68
NT = [(0, 128), (128, 68)]

LAST_RESULT = {}  # test harness peeks at timing info here


def _build_nc(bc):
    """Build the single-core program for bc batches (bc*196 tokens)."""
    T = bc * N
    assert T % 392 == 0
    nch = T // 392

    nc = bacc.Bacc("TRN2", target_bir_lowering=False, debug=False, num_devices=N_CORES)

    xt_d = nc.dram_tensor("xt", [DIM, T], BF16, kind="ExternalInput")
    wqk_d = nc.dram_tensor("wqk", [4, DIM, 128], BF16, kind="ExternalInput")
    bqk_d = nc.dram_tensor("bqk", [128, 4], F32, kind="ExternalInput")
    wv_d = nc.dram_tensor("wv", [DIM, DH], BF16, kind="ExternalInput")
    vb_d = nc.dram_tensor("vb", [128, DH], F32, kind="ExternalInput")
    vbc_d = nc.dram_tensor("vbc", [128, NH], F32, kind="ExternalInput")
    wp_d = nc.dram_tensor("wp", [DH, DIM], BF16, kind="ExternalInput")
    pbr_d = nc.dram_tensor("pbr", [1, DIM], BF16, kind="ExternalInput")
    one_d = nc.dram_tensor("ones", [1, 128], BF16, kind="ExternalInput")
    onem_d = nc.dram_tensor("onem", [128, 128], BF16, kind="ExternalInput")
    ebt_d = nc.dram_tensor("ebt", [128, NH // 2, 1024], BF16, kind="ExternalInput")
    out_d = nc.dram_tensor("out", [T, DIM], F32, kind="ExternalOutput")

    with tile.TileContext(nc) as tc:
        with (
            tc.tile_pool(name="const", bufs=1) as cpool,
            tc.tile_pool(name="qkt", bufs=1) as qpool,
            tc.tile_pool(name="vtile", bufs=4) as vpool,
            tc.tile_pool(name="pu", bufs=3) as pupool,
            tc.tile_pool(name="pb2", bufs=4) as pbpool,
            tc.tile_pool(name="rho", bufs=4) as rpool,
            tc.tile_pool(name="onm", bufs=3) as onpool,
            tc.tile_pool(name="atc", bufs=3) as atpool,
            tc.tile_pool(name="ob", bufs=3) as opool,
            tc.tile_pool(name="mmps", bufs=2, space="PSUM") as mmps,
            tc.tile_pool(name="sps", bufs=2, space="PSUM") as sps,
            tc.tile_pool(name="otps", bufs=2, space="PSUM") as otps,
        ):
            # ---- constants into SBUF; small tensors first so the first
            #      qkv chunk isn't queued behind megabyte-scale DMAs, and
            #      xt split in halves so chunk 0 lands early ----
            bqk_t = cpool.tile([128, 4], F32)
            nc.sync.dma_start(bqk_t[:], bqk_d[:])
            vb_t = cpool.tile([128, DH], F32)
            nc.sync.dma_start(vb_t[:], vb_d[:])
            vbc_t = cpool.tile([128, NH], F32)
            nc.sync.dma_start(vbc_t[:], vbc_d[:])
            pbr_t = cpool.tile([1, DIM], BF16)
            nc.sync.dma_start(pbr_t[:], pbr_d[:])
            one_t = cpool.tile([1, 128], BF16)
            nc.sync.dma_start(one_t[:], one_d[:])
            onem_t = cpool.tile([128, 128], BF16)
            nc.sync.dma_start(onem_t[:], onem_d[:])
            # consolidated multi-dim DMAs, split across the two HWDGE
            # engines (sync + scalar) so descriptor generation for the
            # first qkv chunk's inputs isn't serialized behind everything
            wqk_t = cpool.tile([128, 4, 3, 128], BF16)
            nc.sync.dma_start(
                wqk_t[:, :, :, :],
                wqk_d[:, :, :].rearrange("m (c p) f -> p m c f", c=3))
            xt_t = cpool.tile([128, 3, T], BF16)
            nc.sync.dma_start(
                xt_t[:, :, 0:T // 4],
                xt_d[:, 0:T // 4].rearrange("(c p) f -> p c f", c=3))
            for f0, f1 in [(T // 4, T // 2), (T // 2, T)]:
                nc.scalar.dma_start(
                    xt_t[:, :, f0:f1],
                    xt_d[:, f0:f1].rearrange("(c p) f -> p c f", c=3))
            wv_t = cpool.tile([128, 3, DH], BF16)
            nc.scalar.dma_start(
                wv_t[:, :, :], wv_d[:, :].rearrange("(c p) f -> p c f", c=3))
            ebt_t = cpool.tile([128, NH // 2, 1024], BF16)
            nc.scalar.dma_start(ebt_t[:], ebt_d[:])
            wp_t = cpool.tile([128, NH, DIM], BF16)
            nc.scalar.dma_start(
                wp_t[:, :, :], wp_d[:, :].rearrange("(k p) f -> p k f", k=6))

            # ---- q/k^T phase: tQ/tK hold q/k of heads 0-3 at partition 32h;
            #      tQ2/tK2 hold heads 4-5 at partition 32(h-4) (rows 0:64) ----
            tQ = qpool.tile([128, T], BF16, tag="tq")
            tK = qpool.tile([128, T], BF16, tag="tk")
            tQ2 = qpool.tile([128, T], BF16, tag="tq2")
            tK2 = qpool.tile([128, T], BF16, tag="tk2")
            qk_dests = [(0, tQ, 128), (1, tK, 128), (2, tQ2, 64), (3, tK2, 64)]

            def emit_qkv_chunk(ch):
                c0 = 392 * ch
                for mt, dest, msz in qk_dests:
                    ps = mmps.tile([128, 392], F32, tag="mm")
                    for ct in range(3):
                        nc.tensor.matmul(
                            ps[0:msz, :],
                            wqk_t[:, mt, ct, 0:msz],
                            xt_t[:, ct, c0:c0 + 392],
                            start=(ct == 0), stop=(ct == 2),
                        )
                    nc.scalar.activation(
                        dest[0:msz, c0:c0 + 392], ps[0:msz, :], AF.Identity,
                        bias=bqk_t[0:msz, mt:mt + 1], scale=1.0,
                    )

            # chunk c feeds batches 2c and 2c+1; 4 chunks of runway up
            # front, the rest interleaved into the batch loop as PE filler
            QKV_AHEAD = 2
            for ch in range(QKV_AHEAD):
                emit_qkv_chunk(ch)

            def emit_s(hp, b0):
                """S^T matmuls for one head pair; head hh at column 512*hh of a
                bank-padded [128,1024] PSUM tile (m-tile mt at 512*hh+196*mt).
                Pairs 0 and 1 are emitted back-to-back so their 4 distinct
                tile_position row groups can run concurrently in the PE."""
                s2 = sps.tile([128, 1024], F32, tag="s")
                for hh in range(2):
                    h = 2 * hp + hh
                    if h < 4:
                        qsrc, ksrc, base = tQ, tK, 32 * h
                    else:
                        qsrc, ksrc, base = tQ2, tK2, 32 * (h - 4)
                    for mt, (m0, msz) in enumerate(NT):
                        nc.tensor.matmul(
                            s2[0:msz, 512 * hh + 196 * mt:512 * hh + 196 * mt + 196],
                            ksrc[base:base + 32, b0 + m0:b0 + m0 + msz],
                            qsrc[base:base + 32, b0:b0 + 196],
                            start=True, stop=True,
                            tile_position=(base, 0),
                        )
                return s2

            def process_pair(hp, s2, v_t, at_b):
                # strided [128, 2, 392] views skip the 392:512 pad columns
                # (pads are never read downstream — den/OT use exact regions)
                s2v = s2[:].rearrange("p (h c) -> p h c", h=2)[:, :, 0:392]
                p_u = pupool.tile([128, 1024], BF16, tag="pu")
                p_uv = p_u[:].rearrange("p (h c) -> p h c", h=2)[:, :, 0:392]
                nc.scalar.activation(p_uv, s2v, AF.Exp)
                # P-hat^T = exp(S^T) * exp(bias)^T
                pb2 = pbpool.tile([128, 1024], BF16, tag="pb")
                ebv = ebt_t[:, hp, :].rearrange("p (h c) -> p h c", h=2)[:, :, 0:392]
                nc.vector.tensor_mul(
                    pb2[:].rearrange("p (h c) -> p h c", h=2)[:, :, 0:392],
                    p_uv, ebv)
                # den[n] = sum_m P-hat^T[m, n] via all-ones matmul; the
                # [msz,128] ones stationary broadcasts den to all 128
                # partitions so the normalize multiply needs no
                # partition-broadcast AP (DVE requires nonzero step).
                # One matmul per m-tile covers BOTH heads via a strided
                # rhs AP [msz, 2, 196] (head stride 512).
                # den lands in ot_ps first; the O^T matmuls overwrite it
                # after the reciprocal is taken (WAR order via Tile).
                ot_ps = otps.tile([128, 392], F32, tag="ot")
                pb2v = pb2[:].rearrange("p (h c) -> p h c", h=2)
                for mt, (m0, msz) in enumerate(NT):
                    nc.tensor.matmul(
                        ot_ps[:, :],
                        onem_t[0:msz, :],
                        pb2v[0:msz, :, 196 * mt:196 * mt + 196],
                        start=(mt == 0), stop=(mt == 1),
                    )
                rho = rpool.tile([128, 392], F32, tag="rho")
                nc.vector.reciprocal_approx_fast(rho[:, :], ot_ps[:, :])
                # ---- O^T [128, 196] per head = v^T @ P-hat^T ----
                for hh in range(2):
                    h = 2 * hp + hh
                    for mt, (m0, msz) in enumerate(NT):
                        nc.tensor.matmul(
                            ot_ps[:, 196 * hh:196 * hh + 196],
                            v_t[0:msz, mt, 128 * h:128 * h + 128],
                            pb2[0:msz, 512 * hh + 196 * mt:512 * hh + 196 * mt + 196],
                            start=(mt == 0), stop=(mt == 1),
                        )
                # ---- normalize by rho; heads 3-5 add the v-bias here
                #      (per-partition in O^T layout; softmax rows sum to 1
                #      so O = P(xWv)/den + bv) ----
                o_n = onpool.tile([128, 392], BF16, tag="on")
                nc.vector.tensor_mul(o_n[:, :], ot_ps[:, :], rho[:, :])
                for hh in range(2):
                    h = 2 * hp + hh
                    if h >= 3:
                        reg = slice(196 * hh, 196 * hh + 196)
                        nc.vector.tensor_scalar_add(
                            o_n[:, reg], o_n[:, reg], vbc_t[:, h:h + 1])
                # ---- 6*hardswish(O) = O*(clamp(O,-3,3)+3); the /6 is
                #      folded into the projection weights ----
                m_t = onpool.tile([128, 392], BF16, tag="mt")
                nc.vector.tensor_scalar(
                    out=m_t[:, :], in0=o_n[:, :],
                    scalar1=3.0, scalar2=-3.0, op0=OP.min, op1=OP.max,
                )
                nc.vector.tensor_scalar_add(m_t[:, :], m_t[:, :], 3.0)
                nc.vector.tensor_mul(
                    at_b[:, 392 * hp:392 * hp + 392], m_t[:, :], o_n[:, :])

            def emit_proj(b, at_b):
                b0 = b * N
                for nt, (r0, nsz) in enumerate(NT):
                    ps = mmps.tile([128, 392], F32, tag="mm")
                    nc.tensor.matmul(
                        ps[0:nsz, 0:384], one_t[0:1, 0:nsz], pbr_t[0:1, :],
                        start=True, stop=False,
                    )
                    for kt in range(NH):
                        nc.tensor.matmul(
                            ps[0:nsz, 0:384],
                            at_b[:, 196 * kt + r0:196 * kt + r0 + nsz], wp_t[:, kt, :],
                            start=False, stop=(kt == NH - 1),
                        )
                    ob = opool.tile([128, DIM], F32, tag="ob")
                    nc.scalar.activation(ob[0:nsz, :], ps[0:nsz, 0:384], AF.Copy)
                    nc.sync.dma_start(out_d[b0 + r0:b0 + r0 + nsz, :], ob[0:nsz, :])

            for b in range(bc):
                b0 = b * N
                # ---- v natural [tokens, 768] for this batch; low half
                #      evacuated on DVE (with bias add), high half on ACT
                #      (bias for heads 3-5 folded into o_n above) ----
                v_t = vpool.tile([128, 2, DH], BF16, tag="v")
                for nt, (r0, nsz) in enumerate(NT):
                    for half in range(2):
                        h0 = 384 * half
                        ps = mmps.tile([128, 392], F32, tag="mm")
                        for ct in range(3):
                            nc.tensor.matmul(
                                ps[0:nsz, 0:384],
                                xt_t[:, ct, b0 + r0:b0 + r0 + nsz],
                                wv_t[:, ct, h0:h0 + 384],
                                start=(ct == 0), stop=(ct == 2),
                            )
                        if half == 0:
                            nc.vector.tensor_add(
                                v_t[0:nsz, nt, 0:384], ps[0:nsz, 0:384],
                                vb_t[0:nsz, 0:384],
                            )
                        else:
                            nc.scalar.activation(
                                v_t[0:nsz, nt, 384:768], ps[0:nsz, 0:384], AF.Copy)

                at_b = atpool.tile([128, NH * 196], BF16, tag="at")
                s2a = emit_s(0, b0)
                s2b = emit_s(1, b0)
                # proj for the PREVIOUS batch goes here: its 14 N=384 matmuls
                # fill the PE bubble while this batch's exp/bias-mul run
                if b > 0:
                    emit_proj(b - 1, prev_at)
                if b % 2 == 0 and b // 2 + QKV_AHEAD < nch:
                    emit_qkv_chunk(b // 2 + QKV_AHEAD)
                process_pair(0, s2a, v_t, at_b)
                process_pair(1, s2b, v_t, at_b)
                s2c = emit_s(2, b0)
                process_pair(2, s2c, v_t, at_b)
                prev_at = at_b
            emit_proj(bc - 1, prev_at)

    nc.finalize()  # run Bacc passes (reg alloc, wait splitting) before walrus
    return nc


def _host_pack(x, qkv_w, qkv_b, proj_w, proj_b, attn_biases, bias_idxs, bc):
    """Build the common (replicated) input map and per-core xt slices."""
    w = np.asarray(qkv_w, np.float32).reshape(NH, 192, DIM)
    bia = np.asarray(qkv_b, np.float32).reshape(NH, 192)
    qw = w[:, 0:KD, :] * SCALE          # [6, 32, 384]
    kw = w[:, KD:2 * KD, :]
    vw = w[:, 2 * KD:, :]               # [6, 128, 384]
    qb = bia[:, 0:KD] * SCALE
    kb = bia[:, KD:2 * KD]
    vb = bia[:, 2 * KD:]

    wqk = np.zeros((4, DIM, 128), np.float32)
    wqk[0, :, :] = qw[0:4].reshape(128, DIM).T
    wqk[1, :, :] = kw[0:4].reshape(128, DIM).T
    wqk[2, :, 0:64] = qw[4:6].reshape(64, DIM).T
    wqk[3, :, 0:64] = kw[4:6].reshape(64, DIM).T
    bqk = np.zeros((128, 4), np.float32)
    bqk[:, 0] = qb[0:4].reshape(128)
    bqk[:, 1] = kb[0:4].reshape(128)
    bqk[0:64, 2] = qb[4:6].reshape(64)
    bqk[0:64, 3] = kb[4:6].reshape(64)

    wv = vw.reshape(DH, DIM).T.copy()          # [384, 768], head h at cols 128h
    vbt = np.tile(vb.reshape(1, DH), (128, 1)).astype(np.float32)
    vbc = np.ascontiguousarray(vb.T)           # [128, 6], col h = head h's bias
    # device computes 6*hardswish; absorb the 1/6 into the projection weights
    wp = (np.asarray(proj_w, np.float32).T / 6.0).copy()  # [768, 384]

    bmat = np.asarray(attn_biases, np.float32)[:, np.asarray(bias_idxs)]  # [6,196,196]
    ebp = np.zeros((128, NH // 2, 1024), np.float32)
    ebT = np.exp(np.transpose(bmat, (0, 2, 1)))  # [6, m, n]
    for h in range(NH):
        hp, hh = divmod(h, 2)
        ebp[0:128, hp, 512 * hh + 0:512 * hh + 196] = ebT[h, 0:128, :]
        ebp[0:68, hp, 512 * hh + 196:512 * hh + 392] = ebT[h, 128:196, :]

    common = {
        "pbr": np.asarray(proj_b, np.float32).reshape(1, DIM).astype(NPBF16),
        "ones": np.ones((1, 128), NPBF16),
        "onem": np.ones((128, 128), NPBF16),
        "wqk": wqk.astype(NPBF16),
        "bqk": bqk,
        "wv": wv.astype(NPBF16),
        "vb": vbt,
        "vbc": vbc.astype(np.float32),
        "wp": wp.astype(NPBF16),
        "ebt": ebp.astype(NPBF16),
    }

    x = np.asarray(x, np.float32)
    n_cores = x.shape[0] // bc
    xts = []
    for c in range(n_cores):
        xc = x[bc * c:bc * (c + 1)].reshape(bc * N, DIM)
        xts.append(np.ascontiguousarray(xc.T).astype(NPBF16))
    return common, xts


_NC_CACHE = {}


def kernel(x, qkv_w, qkv_b, proj_w, proj_b, attn_biases, bias_idxs):
    bc = B // N_CORES
    if bc not in _NC_CACHE:
        _NC_CACHE[bc] = _build_nc(bc)
    nc = _NC_CACHE[bc]
    common, xts = _host_pack(x, qkv_w, qkv_b, proj_w, proj_b, attn_biases, bias_idxs, bc)
    in_maps = [dict(common, xt=xts[c]) for c in range(N_CORES)]
    trace = bool(int(os.environ.get("KT_TRACE", "0")))
    res = run_bass_kernel_spmd(nc, in_maps, list(range(N_CORES)), trace=trace)
    LAST_RESULT["exec_time_ns"] = res.exec_time_ns
    LAST_RESULT["mean_exec_time_ns"] = res.mean_exec_time_ns
    outs = [res.results[c]["out"].reshape(bc, N, DIM) for c in range(N_CORES)]
    return np.concatenate(outs, axis=0).astype(np.float32)


# revision 49
# speedup vs baseline: 213.3698x; 1.0088x over previous
"""LeViT-style attention block kernel for Trainium2 (8 NeuronCores, data-parallel over batch).

Reference computation (per batch b of 256, N=196 tokens, DIM=384):
  qkv = x @ qkv_w.T + qkv_b                      [196, 1152]
  q,k,v per head h (6): q,k [196,32], v [196,128]
  S = q @ k.T * 32^-0.5 + bias_h                 [196, 196]
  P = softmax(S, -1)
  O = P @ v  (concat heads -> [196, 768])
  A = hardswish(O)
  out = A @ proj_w.T + proj_b                    [196, 384]

Device mapping (per core: 32 batches = 6272 tokens):
  - host prepacks x.T (bf16), weight tiles, exp(bias)^T tables
  - q/k computed transposed ([head-packed 128 rows, tokens]) on PE,
    interleaved into the batch loop (2 chunks of runway) as PE filler
  - v computed natural per batch ([tokens, 768])
  - S^T[m,n] = k^T q directly on PE (keys m on partitions, queries n free)
    -> exp on ACT -> *exp(bias)^T on DVE = P-hat^T
    -> den[n] = all-ones matmul over P-hat^T rows (PE; [msz,128] ones
       stationary broadcasts den to all partitions), rho = 1/den via
       reciprocal_approx_fast on DVE
    -> O^T = v^T @ P-hat^T on PE (v natural is already the right stationary)
    -> normalize O^T by rho + hardswish on DVE; v-bias of heads 3-5 folded
       here per-partition (softmax rows sum to 1, so O = P(xWv)/den + bv)
    -> A^T per head -> proj matmul (lagged one batch as PE filler) -> out.
  No transpose matmuls and no GpSimd work (the v1 kernel spent 70% of the
  span in GpSimd diag builds for a P^T-via-matmul transpose: 885us; this
  version measures ~345-358us NTFF HW exec across runs).
"""

import os
import sys

import numpy as np

sys.path.insert(0, "/opt/trn_rl_repo")

import ml_dtypes  # noqa: E402

import concourse.bass as bass  # noqa: E402
import concourse.tile as tile  # noqa: E402
from concourse import bacc, mybir  # noqa: E402
from concourse.bass_utils import run_bass_kernel_spmd  # noqa: E402

BF16 = mybir.dt.bfloat16
F32 = mybir.dt.float32
F8 = mybir.dt.float8e4
NPBF16 = ml_dtypes.bfloat16
NPF8 = mybir.dt.np(F8)
DR = mybir.MatmulPerfMode.DoubleRow

N_CORES = 8
B, N, DIM = 256, 196, 384
KD, NH, D = 32, 6, 128  # key dim, heads, per-head v dim
DH = D * NH  # 768
RES = 14
SCALE = KD ** -0.5

AF = mybir.ActivationFunctionType
OP = mybir.AluOpType

# per-batch key/query token tiling: 196 rows split 128 + 68
NT = [(0, 128), (128, 68)]

LAST_RESULT = {}  # test harness peeks at timing info here


def _build_nc(bc):
    """Build the single-core program for bc batches (bc*196 tokens)."""
    T = bc * N
    assert T % 392 == 0
    nch = T // 392

    nc = bacc.Bacc("TRN2", target_bir_lowering=False, debug=False, num_devices=N_CORES)

    xt_d = nc.dram_tensor("xt", [DIM, T], BF16, kind="ExternalInput")
    wqk_d = nc.dram_tensor("wqk", [4, DIM, 128], BF16, kind="ExternalInput")
    bqk_d = nc.dram_tensor("bqk", [128, 4], F32, kind="ExternalInput")
    wv_d = nc.dram_tensor("wv", [DIM, DH], BF16, kind="ExternalInput")
    vb_d = nc.dram_tensor("vb", [128, DH], F32, kind="ExternalInput")
    vbc_d = nc.dram_tensor("vbc", [128, NH], F32, kind="ExternalInput")
    wp_d = nc.dram_tensor("wp", [DH, DIM], BF16, kind="ExternalInput")
    pbr_d = nc.dram_tensor("pbr", [1, DIM], BF16, kind="ExternalInput")
    one_d = nc.dram_tensor("ones", [1, 128], BF16, kind="ExternalInput")
    onem_d = nc.dram_tensor("onem", [128, 128], BF16, kind="ExternalInput")
    ebt_d = nc.dram_tensor("ebt", [128, NH // 2, 1024], BF16, kind="ExternalInput")
    out_d = nc.dram_tensor("out", [T, DIM], F32, kind="ExternalOutput")

    with tile.TileContext(nc) as tc:
        with (
            tc.tile_pool(name="const", bufs=1) as cpool,
            tc.tile_pool(name="qkt", bufs=1) as qpool,
            tc.tile_pool(name="vtile", bufs=4) as vpool,
            tc.tile_pool(name="pu", bufs=3) as pupool,
            tc.tile_pool(name="pb2", bufs=4) as pbpool,
            tc.tile_pool(name="rho", bufs=4) as rpool,
            tc.tile_pool(name="onm", bufs=3) as onpool,
            tc.tile_pool(name="atc", bufs=3) as atpool,
            tc.tile_pool(name="ob", bufs=3) as opool,
            tc.tile_pool(name="mmps", bufs=2, space="PSUM") as mmps,
            tc.tile_pool(name="sps", bufs=2, space="PSUM") as sps,
            tc.tile_pool(name="otps", bufs=2, space="PSUM") as otps,
        ):
            # ---- constants into SBUF; small tensors first so the first
            #      qkv chunk isn't queued behind megabyte-scale DMAs, and
            #      xt split in halves so chunk 0 lands early ----
            bqk_t = cpool.tile([128, 4], F32)
            nc.sync.dma_start(bqk_t[:], bqk_d[:])
            vb_t = cpool.tile([128, DH], F32)
            nc.sync.dma_start(vb_t[:], vb_d[:])
            vbc_t = cpool.tile([128, NH], F32)
            nc.sync.dma_start(vbc_t[:], vbc_d[:])
            pbr_t = cpool.tile([1, DIM], BF16)
            nc.sync.dma_start(pbr_t[:], pbr_d[:])
            one_t = cpool.tile([1, 128], BF16)
            nc.sync.dma_start(one_t[:], one_d[:])
            onem_t = cpool.tile([128, 128], BF16)
            nc.sync.dma_start(onem_t[:], onem_d[:])
            # consolidated multi-dim DMAs, split across the two HWDGE
            # engines (sync + scalar) so descriptor generation for the
            # first qkv chunk's inputs isn't serialized behind everything
            wqk_t = cpool.tile([128, 4, 3, 128], BF16)
            nc.sync.dma_start(
                wqk_t[:, :, :, :],
                wqk_d[:, :, :].rearrange("m (c p) f -> p m c f", c=3))
            xt_t = cpool.tile([128, 3, T], BF16)
            nc.sync.dma_start(
                xt_t[:, :, 0:T // 4],
                xt_d[:, 0:T // 4].rearrange("(c p) f -> p c f", c=3))
            for f0, f1 in [(T // 4, T // 2), (T // 2, T)]:
                nc.scalar.dma_start(
                    xt_t[:, :, f0:f1],
                    xt_d[:, f0:f1].rearrange("(c p) f -> p c f", c=3))
            wv_t = cpool.tile([128, 3, DH], BF16)
            nc.scalar.dma_start(
                wv_t[:, :, :], wv_d[:, :].rearrange("(c p) f -> p c f", c=3))
            ebt_t = cpool.tile([128, NH // 2, 1024], BF16)
            nc.scalar.dma_start(ebt_t[:], ebt_d[:])
            wp_t = cpool.tile([128, NH, DIM], BF16)
            nc.scalar.dma_start(
                wp_t[:, :, :], wp_d[:, :].rearrange("(k p) f -> p k f", k=6))

            # ---- q/k^T phase: tQ/tK hold q/k of heads 0-3 at partition 32h;
            #      tQ2/tK2 hold heads 4-5 at partition 32(h-4) (rows 0:64) ----
            tQ = qpool.tile([128, T], BF16, tag="tq")
            tK = qpool.tile([128, T], BF16, tag="tk")
            tQ2 = qpool.tile([128, T], BF16, tag="tq2")
            tK2 = qpool.tile([128, T], BF16, tag="tk2")
            qk_dests = [(0, tQ, 128), (1, tK, 128), (2, tQ2, 64), (3, tK2, 64)]

            def emit_qkv_chunk(ch):
                c0 = 392 * ch
                for mt, dest, msz in qk_dests:
                    ps = mmps.tile([128, 392], F32, tag="mm")
                    for ct in range(3):
                        nc.tensor.matmul(
                            ps[0:msz, :],
                            wqk_t[:, mt, ct, 0:msz],
                            xt_t[:, ct, c0:c0 + 392],
                            start=(ct == 0), stop=(ct == 2),
                        )
                    nc.scalar.activation(
                        dest[0:msz, c0:c0 + 392], ps[0:msz, :], AF.Identity,
                        bias=bqk_t[0:msz, mt:mt + 1], scale=1.0,
                    )

            # chunk c feeds batches 2c and 2c+1; 4 chunks of runway up
            # front, the rest interleaved into the batch loop as PE filler
            QKV_AHEAD = 2
            for ch in range(QKV_AHEAD):
                emit_qkv_chunk(ch)

            def emit_s(hp, b0):
                """S^T matmuls for one head pair; head hh at column 512*hh of a
                bank-padded [128,1024] PSUM tile (m-tile mt at 512*hh+196*mt).
                Pairs 0 and 1 are emitted back-to-back so their 4 distinct
                tile_position row groups can run concurrently in the PE."""
                s2 = sps.tile([128, 1024], F32, tag="s")
                # mt-major order so consecutive matmuls hit different
                # tile_position row groups (they overlap in the array)
                for mt, (m0, msz) in enumerate(NT):
                    for hh in range(2):
                        h = 2 * hp + hh
                        if h < 4:
                            qsrc, ksrc, base = tQ, tK, 32 * h
                        else:
                            qsrc, ksrc, base = tQ2, tK2, 32 * (h - 4)
                        nc.tensor.matmul(
                            s2[0:msz, 512 * hh + 196 * mt:512 * hh + 196 * mt + 196],
                            ksrc[base:base + 32, b0 + m0:b0 + m0 + msz],
                            qsrc[base:base + 32, b0:b0 + 196],
                            start=True, stop=True,
                            tile_position=(base, 0),
                        )
                return s2

            def process_pair(hp, s2, v_t, at_b):
                # strided [128, 2, 392] views skip the 392:512 pad columns
                # (pads are never read downstream — den/OT use exact regions)
                s2v = s2[:].rearrange("p (h c) -> p h c", h=2)[:, :, 0:392]
                p_u = pupool.tile([128, 1024], BF16, tag="pu")
                p_uv = p_u[:].rearrange("p (h c) -> p h c", h=2)[:, :, 0:392]
                nc.scalar.activation(p_uv, s2v, AF.Exp)
                # P-hat^T = exp(S^T) * exp(bias)^T
                pb2 = pbpool.tile([128, 1024], BF16, tag="pb")
                ebv = ebt_t[:, hp, :].rearrange("p (h c) -> p h c", h=2)[:, :, 0:392]
                nc.vector.tensor_mul(
                    pb2[:].rearrange("p (h c) -> p h c", h=2)[:, :, 0:392],
                    p_uv, ebv)
                # den[n] = sum_m P-hat^T[m, n] via all-ones matmul; the
                # [msz,128] ones stationary broadcasts den to all 128
                # partitions so the normalize multiply needs no
                # partition-broadcast AP (DVE requires nonzero step).
                # One matmul per m-tile covers BOTH heads via a strided
                # rhs AP [msz, 2, 196] (head stride 512).
                # den lands in ot_ps first; the O^T matmuls overwrite it
                # after the reciprocal is taken (WAR order via Tile).
                ot_ps = otps.tile([128, 392], F32, tag="ot")
                pb2v = pb2[:].rearrange("p (h c) -> p h c", h=2)
                for mt, (m0, msz) in enumerate(NT):
                    nc.tensor.matmul(
                        ot_ps[:, :],
                        onem_t[0:msz, :],
                        pb2v[0:msz, :, 196 * mt:196 * mt + 196],
                        start=(mt == 0), stop=(mt == 1),
                    )
                rho = rpool.tile([128, 392], F32, tag="rho")
                nc.vector.reciprocal_approx_fast(rho[:, :], ot_ps[:, :])
                # ---- O^T [128, 196] per head = v^T @ P-hat^T ----
                for hh in range(2):
                    h = 2 * hp + hh
                    for mt, (m0, msz) in enumerate(NT):
                        nc.tensor.matmul(
                            ot_ps[:, 196 * hh:196 * hh + 196],
                            v_t[0:msz, mt, 128 * h:128 * h + 128],
                            pb2[0:msz, 512 * hh + 196 * mt:512 * hh + 196 * mt + 196],
                            start=(mt == 0), stop=(mt == 1),
                        )
                # ---- normalize by rho; heads 3-5 add the v-bias here
                #      (per-partition in O^T layout; softmax rows sum to 1
                #      so O = P(xWv)/den + bv) ----
                o_n = onpool.tile([128, 392], BF16, tag="on")
                nc.vector.tensor_mul(o_n[:, :], ot_ps[:, :], rho[:, :])
                for hh in range(2):
                    h = 2 * hp + hh
                    if h >= 3:
                        reg = slice(196 * hh, 196 * hh + 196)
                        nc.vector.tensor_scalar_add(
                            o_n[:, reg], o_n[:, reg], vbc_t[:, h:h + 1])
                # ---- 6*hardswish(O) = O*(clamp(O,-3,3)+3); the /6 is
                #      folded into the projection weights ----
                m_t = onpool.tile([128, 392], BF16, tag="mt")
                nc.vector.tensor_scalar(
                    out=m_t[:, :], in0=o_n[:, :],
                    scalar1=3.0, scalar2=-3.0, op0=OP.min, op1=OP.max,
                )
                nc.vector.tensor_scalar_add(m_t[:, :], m_t[:, :], 3.0)
                nc.vector.tensor_mul(
                    at_b[:, 392 * hp:392 * hp + 392], m_t[:, :], o_n[:, :])

            def emit_proj(b, at_b):
                b0 = b * N
                for nt, (r0, nsz) in enumerate(NT):
                    ps = mmps.tile([128, 392], F32, tag="mm")
                    nc.tensor.matmul(
                        ps[0:nsz, 0:384], one_t[0:1, 0:nsz], pbr_t[0:1, :],
                        start=True, stop=False,
                    )
                    for kt in range(NH):
                        nc.tensor.matmul(
                            ps[0:nsz, 0:384],
                            at_b[:, 196 * kt + r0:196 * kt + r0 + nsz], wp_t[:, kt, :],
                            start=False, stop=(kt == NH - 1),
                        )
                    ob = opool.tile([128, DIM], F32, tag="ob")
                    nc.scalar.activation(ob[0:nsz, :], ps[0:nsz, 0:384], AF.Copy)
                    nc.sync.dma_start(out_d[b0 + r0:b0 + r0 + nsz, :], ob[0:nsz, :])

            for b in range(bc):
                b0 = b * N
                # ---- v natural [tokens, 768] for this batch; low half
                #      evacuated on DVE (with bias add), high half on ACT
                #      (bias for heads 3-5 folded into o_n above) ----
                v_t = vpool.tile([128, 2, DH], BF16, tag="v")
                for nt, (r0, nsz) in enumerate(NT):
                    for half in range(2):
                        h0 = 384 * half
                        ps = mmps.tile([128, 392], F32, tag="mm")
                        for ct in range(3):
                            nc.tensor.matmul(
                                ps[0:nsz, 0:384],
                                xt_t[:, ct, b0 + r0:b0 + r0 + nsz],
                                wv_t[:, ct, h0:h0 + 384],
                                start=(ct == 0), stop=(ct == 2),
                            )
                        if half == 0:
                            nc.vector.tensor_add(
                                v_t[0:nsz, nt, 0:384], ps[0:nsz, 0:384],
                                vb_t[0:nsz, 0:384],
                            )
                        else:
                            nc.scalar.activation(
                                v_t[0:nsz, nt, 384:768], ps[0:nsz, 0:384], AF.Copy)

                at_b = atpool.tile([128, NH * 196], BF16, tag="at")
                s2a = emit_s(0, b0)
                s2b = emit_s(1, b0)
                # proj for the PREVIOUS batch goes here: its 14 N=384 matmuls
                # fill the PE bubble while this batch's exp/bias-mul run
                if b > 0:
                    emit_proj(b - 1, prev_at)
                if b % 2 == 0 and b // 2 + QKV_AHEAD < nch:
                    emit_qkv_chunk(b // 2 + QKV_AHEAD)
                process_pair(0, s2a, v_t, at_b)
                process_pair(1, s2b, v_t, at_b)
                s2c = emit_s(2, b0)
                process_pair(2, s2c, v_t, at_b)
                prev_at = at_b
            emit_proj(bc - 1, prev_at)

    nc.finalize()  # run Bacc passes (reg alloc, wait splitting) before walrus
    return nc


def _host_pack(x, qkv_w, qkv_b, proj_w, proj_b, attn_biases, bias_idxs, bc):
    """Build the common (replicated) input map and per-core xt slices."""
    w = np.asarray(qkv_w, np.float32).reshape(NH, 192, DIM)
    bia = np.asarray(qkv_b, np.float32).reshape(NH, 192)
    qw = w[:, 0:KD, :] * SCALE          # [6, 32, 384]
    kw = w[:, KD:2 * KD, :]
    vw = w[:, 2 * KD:, :]               # [6, 128, 384]
    qb = bia[:, 0:KD] * SCALE
    kb = bia[:, KD:2 * KD]
    vb = bia[:, 2 * KD:]

    wqk = np.zeros((4, DIM, 128), np.float32)
    wqk[0, :, :] = qw[0:4].reshape(128, DIM).T
    wqk[1, :, :] = kw[0:4].reshape(128, DIM).T
    wqk[2, :, 0:64] = qw[4:6].reshape(64, DIM).T
    wqk[3, :, 0:64] = kw[4:6].reshape(64, DIM).T
    bqk = np.zeros((128, 4), np.float32)
    bqk[:, 0] = qb[0:4].reshape(128)
    bqk[:, 1] = kb[0:4].reshape(128)
    bqk[0:64, 2] = qb[4:6].reshape(64)
    bqk[0:64, 3] = kb[4:6].reshape(64)

    wv = vw.reshape(DH, DIM).T.copy()          # [384, 768], head h at cols 128h
    vbt = np.tile(vb.reshape(1, DH), (128, 1)).astype(np.float32)
    vbc = np.ascontiguousarray(vb.T)           # [128, 6], col h = head h's bias
    # device computes 6*hardswish; absorb the 1/6 into the projection weights
    wp = (np.asarray(proj_w, np.float32).T / 6.0).copy()  # [768, 384]

    bmat = np.asarray(attn_biases, np.float32)[:, np.asarray(bias_idxs)]  # [6,196,196]
    ebp = np.zeros((128, NH // 2, 1024), np.float32)
    ebT = np.exp(np.transpose(bmat, (0, 2, 1)))  # [6, m, n]
    for h in range(NH):
        hp, hh = divmod(h, 2)
        ebp[0:128, hp, 512 * hh + 0:512 * hh + 196] = ebT[h, 0:128, :]
        ebp[0:68, hp, 512 * hh + 196:512 * hh + 392] = ebT[h, 128:196, :]

    common = {
        "pbr": np.asarray(proj_b, np.float32).reshape(1, DIM).astype(NPBF16),
        "ones": np.ones((1, 128), NPBF16),
        "onem": np.ones((128, 128), NPBF16),
        "wqk": wqk.astype(NPBF16),
        "bqk": bqk,
        "wv": wv.astype(NPBF16),
        "vb": vbt,
        "vbc": vbc.astype(np.float32),
        "wp": wp.astype(NPBF16),
        "ebt": ebp.astype(NPBF16),
    }

    x = np.asarray(x, np.float32)
    n_cores = x.shape[0] // bc
    xts = []
    for c in range(n_cores):
        xc = x[bc * c:bc * (c + 1)].reshape(bc * N, DIM)
        xts.append(np.ascontiguousarray(xc.T).astype(NPBF16))
    return common, xts


_NC_CACHE = {}


def kernel(x, qkv_w, qkv_b, proj_w, proj_b, attn_biases, bias_idxs):
    bc = B // N_CORES
    if bc not in _NC_CACHE:
        _NC_CACHE[bc] = _build_nc(bc)
    nc = _NC_CACHE[bc]
    common, xts = _host_pack(x, qkv_w, qkv_b, proj_w, proj_b, attn_biases, bias_idxs, bc)
    in_maps = [dict(common, xt=xts[c]) for c in range(N_CORES)]
    trace = bool(int(os.environ.get("KT_TRACE", "0")))
    res = run_bass_kernel_spmd(nc, in_maps, list(range(N_CORES)), trace=trace)
    LAST_RESULT["exec_time_ns"] = res.exec_time_ns
    LAST_RESULT["mean_exec_time_ns"] = res.mean_exec_time_ns
    outs = [res.results[c]["out"].reshape(bc, N, DIM) for c in range(N_CORES)]
    return np.concatenate(outs, axis=0).astype(np.float32)
